# revision 12
# baseline (speedup 1.0000x reference)
"""MoE forward (top-2 routed, 8 experts) on 8 Trainium2 NeuronCores.

Strategy: expert-parallel. Host computes the (cheap) routing decisions and
gathers each expert's assigned tokens; core e runs its expert's FFN over its
C gathered tokens (fp32r matmuls, fp32 accumulate), computes the gating
softmax on device (fp32) for the gate coefficients, scatters p*(ffn(x)+b2)
rows into a dense [T, D] buffer, then a cross-core ReduceScatter sums the
two expert contributions per token; each core LayerNorms its 512-token shard.
Host concatenates the 8 shards.

Self-contained: hardcodes the problem shapes; compiles per capacity C.
"""
import numpy as np

import concourse.bass as bass
import concourse.bacc as bacc
import concourse.tile as tile
import concourse.mybir as mybir
import concourse.bass_utils as bass_utils
from concourse.bass import IndirectOffsetOnAxis

P = 128
N_CORES = 8
TEMP = 0.9
EPS = 1e-5

f32 = mybir.dt.float32
f32r = mybir.dt.float32r
i32 = mybir.dt.int32
AF = mybir.ActivationFunctionType
OP = mybir.AluOpType


def _sub_blocks(n, pref=512):
    """Split n (multiple of 128) into matmul free-dim blocks <= 512,
    preferring >=256 (full-rate fp32r)."""
    out = []
    rem = n
    while rem > 0:
        if rem == 128 + pref:
            out += [384, 256]
            rem = 0
        elif rem >= pref:
            out.append(pref)
            rem -= pref
        else:
            out.append(rem)
            rem = 0
    return out


def build(C, D=1024, F=4096, E=8, T=4096):
    """Build the SPMD Bass program for capacity C (multiple of 128)."""
    DC, FC = D // P, F // P
    MG = FC // 4                  # m-groups of 4 F-chunks (w1/w2 panel unit)
    NCk = C // P                  # token chunks per core
    TOK = T // N_CORES            # tokens per output shard
    CHUNK = TOK + 1               # +1 trash row per shard chunk
    ZROWS = N_CORES * CHUNK
    LNC = TOK // P                # LayerNorm tiles per shard
    assert D % P == 0 and F % (4 * P) == 0 and TOK % P == 0 and C % P == 0

    nc = bacc.Bacc("TRN2", target_bir_lowering=False, debug=False,
                   enable_asserts=True, num_devices=N_CORES)

    xT = nc.dram_tensor("xT", [D, C], f32, kind="ExternalInput").ap()
    gw = nc.dram_tensor("gw", [D, E], f32, kind="ExternalInput").ap()
    gb = nc.dram_tensor("gb", [P, E], f32, kind="ExternalInput").ap()
    w1 = nc.dram_tensor("w1", [D, F], f32r, kind="ExternalInput").ap()
    b1 = nc.dram_tensor("b1", [F], f32, kind="ExternalInput").ap()
    w2 = nc.dram_tensor("w2", [F, D], f32r, kind="ExternalInput").ap()
    b2 = nc.dram_tensor("b2", [P, D], f32, kind="ExternalInput").ap()
    dest = nc.dram_tensor("dest", [C], i32, kind="ExternalInput").ap()
    lng = nc.dram_tensor("ln_g", [P, D], f32, kind="ExternalInput").ap()
    lnb = nc.dram_tensor("ln_b", [P, D], f32, kind="ExternalInput").ap()
    out = nc.dram_tensor("out", [TOK, D], f32, kind="ExternalOutput").ap()

    with tile.TileContext(nc) as tc:
        with (
            tc.tile_pool(name="res", bufs=1) as res,          # resident tiles
            tc.tile_pool(name="wpan", bufs=3) as wpan,        # weight panels
            tc.tile_pool(name="hbuf", bufs=1) as hbuf,        # h (gelu out)
            tc.tile_pool(name="work", bufs=2) as work,        # LN / misc
            tc.tile_pool(name="gat", bufs=1) as gat,          # gating temps
            tc.tile_pool(name="ps1", bufs=3, space="PSUM") as ps1p,
            tc.tile_pool(name="ps2", bufs=4, space="PSUM") as ps2p,
            tc.tile_pool(name="dram", bufs=1, space="DRAM") as dram,
        ):
            # ---------------- resident loads ----------------
            x_sb = res.tile([P, DC, C], f32r)
            # gpsimd cast DMA: rounds fp32 -> fp32r once here (PE input prep)
            nc.gpsimd.dma_start(x_sb[:], xT.rearrange("(dc p) c -> p dc c", p=P))
            gw_sb = res.tile([P, DC, E], f32)
            nc.sync.dma_start(gw_sb[:], gw.rearrange("(dc p) e -> p dc e", p=P))
            gb_sb = res.tile([P, E], f32)
            nc.sync.dma_start(gb_sb[:], gb[:, :])
            b1_sb = res.tile([P, FC], f32)
            nc.sync.dma_start(b1_sb[:], b1.rearrange("(fc p) -> p fc", p=P))
            b2_sb = res.tile([P, D], f32)
            nc.sync.dma_start(b2_sb[:], b2[:, :])
            lng_sb = res.tile([P, D], f32)
            nc.sync.dma_start(lng_sb[:], lng[:, :])
            lnb_sb = res.tile([P, D], f32)
            nc.sync.dma_start(lnb_sb[:], lnb[:, :])
            dest_sb = res.tile([P, NCk], i32)
            nc.sync.dma_start(dest_sb[:], dest.rearrange("(c p) -> p c", p=P))
            eps_sb = res.tile([P, 1], f32)
            nc.vector.memset(eps_sb[:], EPS)

            # ---------------- Z dense combine buffer + zero-fill ----------
            Z = dram.tile([ZROWS, D], f32)
            zer = res.tile([P, D], f32)
            nc.vector.memset(zer[:], 0.0)
            r0 = 0
            while r0 < ZROWS:
                rr = min(P, ZROWS - r0)
                nc.sync.dma_start(Z[r0:r0 + rr, :], zer[:rr, :])
                r0 += rr

            # ---------------- gating (fp32) ----------------
            logit = gat.tile([P, NCk, E], f32)
            for t in range(NCk):
                # True-fp32 load of this token chunk straight from DRAM: the
                # fp32r copy x_sb is rounded, and f32r rounding noise in the
                # gating logits can flip top-2 decisions vs the host routing.
                xg = work.tile([P, DC, P], f32, tag="xg")
                nc.sync.dma_start(
                    xg[:],
                    xT.rearrange("(dc p) c -> p dc c", p=P)[:, :, t * P:(t + 1) * P])
                pg = ps1p.tile([P, E], f32, tag="ps1")
                for dc in range(DC):
                    nc.tensor.matmul(
                        pg[:],
                        xg[:, dc, :],
                        gw_sb[:, dc, :],
                        start=(dc == 0), stop=(dc == DC - 1),
                    )
                nc.vector.tensor_add(logit[:, t, :], pg[:], gb_sb[:])

            m1 = gat.tile([P, NCk, 1], f32)
            nc.vector.tensor_reduce(m1[:], logit[:], axis=mybir.AxisListType.X,
                                    op=OP.max)
            tt = gat.tile([P, NCk, E], f32)
            nc.vector.tensor_tensor(tt[:], logit[:],
                                    m1[:].to_broadcast([P, NCk, E]), OP.subtract)
            eq = gat.tile([P, NCk, E], f32)
            nc.vector.tensor_scalar(eq[:], tt[:], 0.0, None, OP.is_equal)
            msk = gat.tile([P, NCk, E], f32)
            nc.vector.scalar_tensor_tensor(msk[:], eq[:], -1e30, tt[:],
                                           OP.mult, OP.add)
            m2 = gat.tile([P, NCk, 1], f32)
            nc.vector.tensor_reduce(m2[:], msk[:], axis=mybir.AxisListType.X,
                                    op=OP.max)
            keep = gat.tile([P, NCk, E], f32)
            nc.vector.tensor_tensor(keep[:], tt[:],
                                    m2[:].to_broadcast([P, NCk, E]), OP.is_ge)
            ex = gat.tile([P, NCk, E], f32)
            nc.scalar.activation(ex[:], tt[:], AF.Exp, scale=float(1.0 / TEMP))
            exk = gat.tile([P, NCk, E], f32)
            nc.vector.tensor_mul(exk[:], ex[:], keep[:])
            ssum = gat.tile([P, NCk, 1], f32)
            nc.vector.tensor_reduce(ssum[:], exk[:], axis=mybir.AxisListType.X,
                                    op=OP.add)
            rcp = gat.tile([P, NCk, 1], f32)
            nc.vector.reciprocal(rcp[:], ssum[:])
            pcols = gat.tile([P, NCk], f32)
            nc.vector.tensor_tensor(pcols[:], exk[:, :, 0],
                                    rcp[:, :, 0], OP.mult)

            # ---------------- y accumulator init: y = p * b2 --------------
            y_acc = res.tile([P, NCk, D], f32)
            for mt in range(NCk):
                nc.vector.tensor_scalar(y_acc[:, mt, :], b2_sb[:],
                                        pcols[:, mt:mt + 1], None, OP.mult)

            # ---------------- main FFN loop ----------------
            nsubs = _sub_blocks(C)
            ndsubs = _sub_blocks(D)
            for mg in range(MG):
                w1p = wpan.tile([P, DC, 4 * P], f32r, tag="wp")
                nc.sync.dma_start(
                    w1p[:],
                    w1[:, mg * 4 * P:(mg + 1) * 4 * P]
                    .rearrange("(dc p) m -> p dc m", p=P))
                w2p = wpan.tile([P, 4, D], f32r, tag="wp")
                nc.sync.dma_start(
                    w2p[:],
                    w2[mg * 4 * P:(mg + 1) * 4 * P, :]
                    .rearrange("(mi p) d -> p mi d", p=P))
                h_mg = hbuf.tile([P, 4, C], f32r, tag="h")
                # mm1: h = gelu(x @ w1 + b1), transposed [F-part, tokens]
                for mi in range(4):
                    c0 = 0
                    for ns in nsubs:
                        ps = ps1p.tile([P, ns], f32, tag="ps1")
                        for dc in range(DC):
                            nc.tensor.matmul(
                                ps[:],
                                w1p[:, dc, mi * P:(mi + 1) * P],
                                x_sb[:, dc, c0:c0 + ns],
                                start=(dc == 0), stop=(dc == DC - 1),
                            )
                        nc.scalar.activation(
                            h_mg[:, mi, c0:c0 + ns], ps[:], AF.Gelu,
                            bias=b1_sb[:, mg * 4 + mi:mg * 4 + mi + 1])
                        c0 += ns
                # mm2: y[tok, d] += p * (h.T @ w2)
                for mt in range(NCk):
                    d0 = 0
                    for nd in ndsubs:
                        ps = ps2p.tile([P, nd], f32, tag="ps2")
                        for mi in range(4):
                            nc.tensor.matmul(
                                ps[:],
                                h_mg[:, mi, mt * P:(mt + 1) * P],
                                w2p[:, mi, d0:d0 + nd],
                                start=(mi == 0), stop=(mi == 3),
                            )
                        nc.vector.scalar_tensor_tensor(
                            y_acc[:, mt, d0:d0 + nd], ps[:],
                            pcols[:, mt:mt + 1], y_acc[:, mt, d0:d0 + nd],
                            OP.mult, OP.add)
                        d0 += nd

            # ---------------- scatter into Z ----------------
            for mt in range(NCk):
                nc.gpsimd.indirect_dma_start(
                    out=Z[:, :],
                    out_offset=IndirectOffsetOnAxis(
                        ap=dest_sb[:, mt:mt + 1], axis=0),
                    in_=y_acc[:, mt, :],
                    in_offset=None,
                )

            # ---------------- cross-core combine ----------------
            rs = dram.tile([CHUNK, D], f32)
            nc.gpsimd.collective_compute(
                "ReduceScatter",
                OP.add,
                replica_groups=[list(range(N_CORES))],
                ins=[Z.opt()],
                outs=[rs.opt()],
            )

            # ---------------- LayerNorm on the shard ----------------
            for ch in range(LNC):
                xt = work.tile([P, D], f32, tag="ln_x")
                nc.sync.dma_start(xt[:], rs[ch * P:(ch + 1) * P, :])
                s1 = work.tile([P, 1], f32, tag="ln_s1")
                nc.vector.tensor_reduce(s1[:], xt[:], axis=mybir.AxisListType.X,
                                        op=OP.add)
                scr = work.tile([P, D], f32, tag="ln_scr")
                s2 = work.tile([P, 1], f32, tag="ln_s2")
                nc.scalar.activation(scr[:], xt[:], AF.Square, accum_out=s2[:])
                mean = work.tile([P, 1], f32, tag="ln_mean")
                nc.vector.tensor_scalar_mul(mean[:], s1[:], 1.0 / D)
                ex2 = work.tile([P, 1], f32, tag="ln_ex2")
                nc.vector.tensor_scalar_mul(ex2[:], s2[:], 1.0 / D)
                # var = ex2 - mean^2  (computed as (-mean)*mean + ex2)
                var = work.tile([P, 1], f32, tag="ln_var")
                nc.vector.scalar_tensor_tensor(var[:], mean[:], -1.0, mean[:],
                                               OP.mult, OP.mult)
                nc.vector.tensor_add(var[:], var[:], ex2[:])
                sd = work.tile([P, 1], f32, tag="ln_sd")
                nc.scalar.activation(sd[:], var[:], AF.Sqrt, bias=eps_sb[:])
                rstd = work.tile([P, 1], f32, tag="ln_rstd")
                nc.vector.reciprocal(rstd[:], sd[:])
                nrm = work.tile([P, D], f32, tag="ln_nrm")
                nc.vector.tensor_scalar(nrm[:], xt[:], mean[:], rstd[:],
                                        OP.subtract, OP.mult)
                nc.vector.tensor_mul(nrm[:], nrm[:], lng_sb[:])
                nc.vector.tensor_add(nrm[:], nrm[:], lnb_sb[:])
                nc.sync.dma_start(out[ch * P:(ch + 1) * P, :], nrm[:])

    nc.compile()
    return nc


_CACHE = {}


def _get_program(C, D, F, E, T):
    key = (C, D, F, E, T)
    if key not in _CACHE:
        _CACHE[key] = build(C, D=D, F=F, E=E, T=T)
    return _CACHE[key]


def prepare(x, gate_w, gate_b, w1, b1, w2, b2, ln_g, ln_b):
    """Host-side routing + per-core input construction.

    Returns (C, dims, in_maps)."""
    x = np.asarray(x, np.float32)
    gate_w = np.asarray(gate_w, np.float32)
    gate_b = np.asarray(gate_b, np.float32)
    w1 = np.asarray(w1, np.float32)
    b1 = np.asarray(b1, np.float32)
    w2 = np.asarray(w2, np.float32)
    b2 = np.asarray(b2, np.float32)
    ln_g = np.asarray(ln_g, np.float32)
    ln_b = np.asarray(ln_b, np.float32)

    Bb, S, D = x.shape
    E, _, F = w1.shape
    T = Bb * S
    TOK = T // N_CORES
    CHUNK = TOK + 1

    # ---- host routing (decisions only; probabilities are computed on device)
    x2 = np.ascontiguousarray(x.reshape(T, D))
    logits = (x2 @ gate_w + gate_b) / TEMP
    top2 = np.argsort(-logits, axis=1, kind="stable")[:, :2]
    idx = [np.where((top2 == e).any(axis=1))[0] for e in range(E)]
    maxc = max(len(i) for i in idx)
    C = max(((maxc + P - 1) // P) * P, P)

    in_maps = []
    for e in range(E):
        ids = idx[e]
        xTe = np.zeros((D, C), np.float32)
        xTe[:, :len(ids)] = x2[ids].T
        de = np.full((C,), TOK, np.int32)  # pad -> trash row (chunk0, row TOK)
        de[:len(ids)] = (ids // TOK) * CHUNK + (ids % TOK)
        perm = np.roll(np.arange(E), -e)   # col 0 = own expert
        in_maps.append({
            "xT": xTe,
            "gw": np.ascontiguousarray(gate_w[:, perm]),
            "gb": np.ascontiguousarray(
                np.broadcast_to(gate_b[perm], (P, E))),
            "w1": np.ascontiguousarray(w1[e]),
            "b1": np.ascontiguousarray(b1[e]),
            "w2": np.ascontiguousarray(w2[e]),
            "b2": np.ascontiguousarray(np.broadcast_to(b2[e], (P, D))),
            "dest": de,
            "ln_g": np.ascontiguousarray(np.broadcast_to(ln_g, (P, D))),
            "ln_b": np.ascontiguousarray(np.broadcast_to(ln_b, (P, D))),
        })

    return C, (Bb, S, D, F, E, T), in_maps


def kernel(x, gate_w, gate_b, w1, b1, w2, b2, ln_g, ln_b):
    C, (Bb, S, D, F, E, T), in_maps = prepare(
        x, gate_w, gate_b, w1, b1, w2, b2, ln_g, ln_b)
    nc = _get_program(C, D, F, E, T)
    res = bass_utils.run_bass_kernel_spmd(
        nc, in_maps, core_ids=list(range(N_CORES)))
    shards = [res.results[c]["out"] for c in range(N_CORES)]
    return np.concatenate(shards, axis=0).reshape(Bb, S, D)


# revision 18
# speedup vs baseline: 1.0270x; 1.0270x over previous
"""MoE forward (top-2 routed, 8 experts) on 8 Trainium2 NeuronCores.

Strategy: expert-parallel. Host computes the (cheap) routing decisions and
gathers each expert's assigned tokens; core e runs its expert's FFN over its
C gathered tokens (fp32r matmuls, fp32 accumulate), computes the gating
softmax on device (fp32) for the gate coefficients, scatters p*(ffn(x)+b2)
rows into a dense [T, D] buffer, then a cross-core ReduceScatter sums the
two expert contributions per token; each core LayerNorms its 512-token shard.
Host concatenates the 8 shards.

Self-contained: hardcodes the problem shapes; compiles per capacity C.
"""
import numpy as np

import concourse.bass as bass
import concourse.bacc as bacc
import concourse.tile as tile
import concourse.mybir as mybir
import concourse.bass_utils as bass_utils
from concourse.bass import IndirectOffsetOnAxis

P = 128
N_CORES = 8
TEMP = 0.9
EPS = 1e-5

f32 = mybir.dt.float32
f32r = mybir.dt.float32r
i32 = mybir.dt.int32
AF = mybir.ActivationFunctionType
OP = mybir.AluOpType


def _sub_blocks(n, pref=512):
    """Split n (multiple of 128) into matmul free-dim blocks <= 512,
    preferring >=256 (full-rate fp32r)."""
    out = []
    rem = n
    while rem > 0:
        if rem == 128 + pref:
            out += [384, 256]
            rem = 0
        elif rem >= pref:
            out.append(pref)
            rem -= pref
        else:
            out.append(rem)
            rem = 0
    return out


def build(C, D=1024, F=4096, E=8, T=4096):
    """Build the SPMD Bass program for capacity C (multiple of 128)."""
    DC, FC = D // P, F // P
    MG = FC // 4                  # m-groups of 4 F-chunks (w1/w2 panel unit)
    NCk = C // P                  # token chunks per core
    TOK = T // N_CORES            # tokens per output shard
    CHUNK = TOK + 1               # +1 trash row per shard chunk
    ZROWS = N_CORES * CHUNK
    LNC = TOK // P                # LayerNorm tiles per shard
    assert D % P == 0 and F % (4 * P) == 0 and TOK % P == 0 and C % P == 0

    nc = bacc.Bacc("TRN2", target_bir_lowering=False, debug=False,
                   enable_asserts=True, num_devices=N_CORES)

    # All big inputs are host pre-tiled to [.., P, free] so each DMA is 128
    # contiguous per-partition descriptors (DMA queues are descriptor-bound
    # otherwise).
    xT = nc.dram_tensor("xT", [P, DC * C], f32, kind="ExternalInput").ap()
    xg_t = nc.dram_tensor("xg_t", [NCk, P, DC * P], f32,
                          kind="ExternalInput").ap()
    gw = nc.dram_tensor("gw", [D, E], f32, kind="ExternalInput").ap()
    gb = nc.dram_tensor("gb", [P, E], f32, kind="ExternalInput").ap()
    w1 = nc.dram_tensor("w1", [MG, P, DC * 4 * P], f32r,
                        kind="ExternalInput").ap()
    b1 = nc.dram_tensor("b1", [F], f32, kind="ExternalInput").ap()
    w2 = nc.dram_tensor("w2", [MG, P, 4 * D], f32r,
                        kind="ExternalInput").ap()
    b2 = nc.dram_tensor("b2", [P, D], f32, kind="ExternalInput").ap()
    dest = nc.dram_tensor("dest", [C], i32, kind="ExternalInput").ap()
    lng = nc.dram_tensor("ln_g", [P, D], f32, kind="ExternalInput").ap()
    lnb = nc.dram_tensor("ln_b", [P, D], f32, kind="ExternalInput").ap()
    out = nc.dram_tensor("out", [TOK, D], f32, kind="ExternalOutput").ap()

    with tile.TileContext(nc) as tc:
        with (
            tc.tile_pool(name="res", bufs=1) as res,          # resident tiles
            tc.tile_pool(name="wpan", bufs=3) as wpan,        # weight panels
            tc.tile_pool(name="hbuf", bufs=1) as hbuf,        # h (gelu out)
            tc.tile_pool(name="work", bufs=2) as work,        # LN / misc
            tc.tile_pool(name="gat", bufs=1) as gat,          # gating temps
            tc.tile_pool(name="ps1", bufs=3, space="PSUM") as ps1p,
            tc.tile_pool(name="ps2", bufs=4, space="PSUM") as ps2p,
            tc.tile_pool(name="dram", bufs=1, space="DRAM") as dram,
        ):
            # ---------------- resident loads ----------------
            x_sb = res.tile([P, DC, C], f32r)
            # gpsimd cast DMA: rounds fp32 -> fp32r once here (PE input prep)
            nc.gpsimd.dma_start(x_sb[:], xT.rearrange("p (dc c) -> p dc c", dc=DC))
            gw_sb = res.tile([P, DC, E], f32)
            nc.sync.dma_start(gw_sb[:], gw.rearrange("(dc p) e -> p dc e", p=P))
            gb_sb = res.tile([P, E], f32)
            nc.sync.dma_start(gb_sb[:], gb[:, :])
            b1_sb = res.tile([P, FC], f32)
            nc.sync.dma_start(b1_sb[:], b1.rearrange("(fc p) -> p fc", p=P))
            b2_sb = res.tile([P, D], f32)
            nc.sync.dma_start(b2_sb[:], b2[:, :])
            lng_sb = res.tile([P, D], f32)
            nc.sync.dma_start(lng_sb[:], lng[:, :])
            lnb_sb = res.tile([P, D], f32)
            nc.sync.dma_start(lnb_sb[:], lnb[:, :])
            dest_sb = res.tile([P, NCk], i32)
            nc.sync.dma_start(dest_sb[:], dest.rearrange("(c p) -> p c", p=P))
            eps_sb = res.tile([P, 1], f32)
            nc.vector.memset(eps_sb[:], EPS)

            # ---------------- Z dense combine buffer + zero-fill ----------
            Z = dram.tile([ZROWS, D], f32)
            zn = ZROWS * D // P // 8  # 8 zero-fill DMAs of [P, zn]
            assert ZROWS * D == P * 8 * zn
            zer = res.tile([P, zn], f32)
            nc.vector.memset(zer[:], 0.0)
            zflat = Z.rearrange("r d -> (r d)")
            for j in range(8):
                nc.sync.dma_start(
                    zflat[j * P * zn:(j + 1) * P * zn]
                    .rearrange("(p f) -> p f", p=P), zer[:])

            # ---------------- gating (fp32) ----------------
            logit = gat.tile([P, NCk, E], f32)
            for t in range(NCk):
                # True-fp32 load of this token chunk straight from DRAM: the
                # fp32r copy x_sb is rounded, and f32r rounding noise in the
                # gating logits can flip top-2 decisions vs the host routing.
                xg = work.tile([P, DC, P], f32, tag="xg")
                nc.sync.dma_start(
                    xg[:], xg_t[t].rearrange("p (dc q) -> p dc q", dc=DC))
                pg = ps1p.tile([P, E], f32, tag="ps1")
                for dc in range(DC):
                    nc.tensor.matmul(
                        pg[:],
                        xg[:, dc, :],
                        gw_sb[:, dc, :],
                        start=(dc == 0), stop=(dc == DC - 1),
                    )
                nc.vector.tensor_add(logit[:, t, :], pg[:], gb_sb[:])

            m1 = gat.tile([P, NCk, 1], f32)
            nc.vector.tensor_reduce(m1[:], logit[:], axis=mybir.AxisListType.X,
                                    op=OP.max)
            tt = gat.tile([P, NCk, E], f32)
            nc.vector.tensor_tensor(tt[:], logit[:],
                                    m1[:].to_broadcast([P, NCk, E]), OP.subtract)
            eq = gat.tile([P, NCk, E], f32)
            nc.vector.tensor_scalar(eq[:], tt[:], 0.0, None, OP.is_equal)
            msk = gat.tile([P, NCk, E], f32)
            nc.vector.scalar_tensor_tensor(msk[:], eq[:], -1e30, tt[:],
                                           OP.mult, OP.add)
            m2 = gat.tile([P, NCk, 1], f32)
            nc.vector.tensor_reduce(m2[:], msk[:], axis=mybir.AxisListType.X,
                                    op=OP.max)
            keep = gat.tile([P, NCk, E], f32)
            nc.vector.tensor_tensor(keep[:], tt[:],
                                    m2[:].to_broadcast([P, NCk, E]), OP.is_ge)
            ex = gat.tile([P, NCk, E], f32)
            nc.scalar.activation(ex[:], tt[:], AF.Exp, scale=float(1.0 / TEMP))
            exk = gat.tile([P, NCk, E], f32)
            nc.vector.tensor_mul(exk[:], ex[:], keep[:])
            ssum = gat.tile([P, NCk, 1], f32)
            nc.vector.tensor_reduce(ssum[:], exk[:], axis=mybir.AxisListType.X,
                                    op=OP.add)
            rcp = gat.tile([P, NCk, 1], f32)
            nc.vector.reciprocal(rcp[:], ssum[:])
            pcols = gat.tile([P, NCk], f32)
            nc.vector.tensor_tensor(pcols[:], exk[:, :, 0],
                                    rcp[:, :, 0], OP.mult)

            # ---------------- y accumulator init: y = p * b2 --------------
            y_acc = res.tile([P, NCk, D], f32)
            for mt in range(NCk):
                nc.vector.tensor_scalar(y_acc[:, mt, :], b2_sb[:],
                                        pcols[:, mt:mt + 1], None, OP.mult)

            # ---------------- main FFN loop ----------------
            nsubs = _sub_blocks(C)
            ndsubs = _sub_blocks(D)
            for mg in range(MG):
                w1p = wpan.tile([P, DC, 4 * P], f32r, tag="wp")
                nc.sync.dma_start(
                    w1p[:], w1[mg].rearrange("p (dc m) -> p dc m", dc=DC))
                w2p = wpan.tile([P, 4, D], f32r, tag="wp")
                nc.sync.dma_start(
                    w2p[:], w2[mg].rearrange("p (mi d) -> p mi d", mi=4))
                h_mg = hbuf.tile([P, 4, C], f32r, tag="h")
                # mm1: h = gelu(x @ w1 + b1), transposed [F-part, tokens]
                for mi in range(4):
                    c0 = 0
                    for ns in nsubs:
                        ps = ps1p.tile([P, ns], f32, tag="ps1")
                        for dc in range(DC):
                            nc.tensor.matmul(
                                ps[:],
                                w1p[:, dc, mi * P:(mi + 1) * P],
                                x_sb[:, dc, c0:c0 + ns],
                                start=(dc == 0), stop=(dc == DC - 1),
                            )
                        nc.scalar.activation(
                            h_mg[:, mi, c0:c0 + ns], ps[:], AF.Gelu,
                            bias=b1_sb[:, mg * 4 + mi:mg * 4 + mi + 1])
                        c0 += ns
                # mm2: y[tok, d] += p * (h.T @ w2)
                for mt in range(NCk):
                    d0 = 0
                    for nd in ndsubs:
                        ps = ps2p.tile([P, nd], f32, tag="ps2")
                        for mi in range(4):
                            nc.tensor.matmul(
                                ps[:],
                                h_mg[:, mi, mt * P:(mt + 1) * P],
                                w2p[:, mi, d0:d0 + nd],
                                start=(mi == 0), stop=(mi == 3),
                            )
                        nc.vector.scalar_tensor_tensor(
                            y_acc[:, mt, d0:d0 + nd], ps[:],
                            pcols[:, mt:mt + 1], y_acc[:, mt, d0:d0 + nd],
                            OP.mult, OP.add)
                        d0 += nd

            # ---------------- scatter into Z ----------------
            for mt in range(NCk):
                nc.gpsimd.indirect_dma_start(
                    out=Z[:, :],
                    out_offset=IndirectOffsetOnAxis(
                        ap=dest_sb[:, mt:mt + 1], axis=0),
                    in_=y_acc[:, mt, :],
                    in_offset=None,
                )

            # ---------------- cross-core combine ----------------
            rs = dram.tile([CHUNK, D], f32)
            nc.gpsimd.collective_compute(
                "ReduceScatter",
                OP.add,
                replica_groups=[list(range(N_CORES))],
                ins=[Z.opt()],
                outs=[rs.opt()],
            )

            # ---------------- LayerNorm on the shard ----------------
            for ch in range(LNC):
                xt = work.tile([P, D], f32, tag="ln_x")
                nc.sync.dma_start(xt[:], rs[ch * P:(ch + 1) * P, :])
                s1 = work.tile([P, 1], f32, tag="ln_s1")
                nc.vector.tensor_reduce(s1[:], xt[:], axis=mybir.AxisListType.X,
                                        op=OP.add)
                scr = work.tile([P, D], f32, tag="ln_scr")
                s2 = work.tile([P, 1], f32, tag="ln_s2")
                nc.scalar.activation(scr[:], xt[:], AF.Square, accum_out=s2[:])
                mean = work.tile([P, 1], f32, tag="ln_mean")
                nc.vector.tensor_scalar_mul(mean[:], s1[:], 1.0 / D)
                ex2 = work.tile([P, 1], f32, tag="ln_ex2")
                nc.vector.tensor_scalar_mul(ex2[:], s2[:], 1.0 / D)
                # var = ex2 - mean^2  (computed as (-mean)*mean + ex2)
                var = work.tile([P, 1], f32, tag="ln_var")
                nc.vector.scalar_tensor_tensor(var[:], mean[:], -1.0, mean[:],
                                               OP.mult, OP.mult)
                nc.vector.tensor_add(var[:], var[:], ex2[:])
                sd = work.tile([P, 1], f32, tag="ln_sd")
                nc.scalar.activation(sd[:], var[:], AF.Sqrt, bias=eps_sb[:])
                rstd = work.tile([P, 1], f32, tag="ln_rstd")
                nc.vector.reciprocal(rstd[:], sd[:])
                nrm = work.tile([P, D], f32, tag="ln_nrm")
                nc.vector.tensor_scalar(nrm[:], xt[:], mean[:], rstd[:],
                                        OP.subtract, OP.mult)
                nc.vector.tensor_mul(nrm[:], nrm[:], lng_sb[:])
                nc.vector.tensor_add(nrm[:], nrm[:], lnb_sb[:])
                nc.sync.dma_start(out[ch * P:(ch + 1) * P, :], nrm[:])

    nc.compile()
    return nc


_CACHE = {}


def _get_program(C, D, F, E, T):
    key = (C, D, F, E, T)
    if key not in _CACHE:
        _CACHE[key] = build(C, D=D, F=F, E=E, T=T)
    return _CACHE[key]


def prepare(x, gate_w, gate_b, w1, b1, w2, b2, ln_g, ln_b):
    """Host-side routing + per-core input construction.

    Returns (C, dims, in_maps)."""
    x = np.asarray(x, np.float32)
    gate_w = np.asarray(gate_w, np.float32)
    gate_b = np.asarray(gate_b, np.float32)
    w1 = np.asarray(w1, np.float32)
    b1 = np.asarray(b1, np.float32)
    w2 = np.asarray(w2, np.float32)
    b2 = np.asarray(b2, np.float32)
    ln_g = np.asarray(ln_g, np.float32)
    ln_b = np.asarray(ln_b, np.float32)

    Bb, S, D = x.shape
    E, _, F = w1.shape
    T = Bb * S
    TOK = T // N_CORES
    CHUNK = TOK + 1

    # ---- host routing (decisions only; probabilities are computed on device)
    x2 = np.ascontiguousarray(x.reshape(T, D))
    logits = (x2 @ gate_w + gate_b) / TEMP
    top2 = np.argsort(-logits, axis=1, kind="stable")[:, :2]
    idx = [np.where((top2 == e).any(axis=1))[0] for e in range(E)]
    maxc = max(len(i) for i in idx)
    C = max(((maxc + P - 1) // P) * P, P)

    DC, NCk, MG = D // P, C // P, F // (4 * P)
    in_maps = []
    for e in range(E):
        ids = idx[e]
        xTe = np.zeros((D, C), np.float32)
        xTe[:, :len(ids)] = x2[ids].T
        xT_t = np.ascontiguousarray(
            xTe.reshape(DC, P, C).transpose(1, 0, 2).reshape(P, DC * C))
        xg_t = np.ascontiguousarray(
            xTe.reshape(DC, P, NCk, P).transpose(2, 1, 0, 3)
            .reshape(NCk, P, DC * P))
        w1t = np.ascontiguousarray(
            w1[e].reshape(DC, P, MG, 4 * P).transpose(2, 1, 0, 3)
            .reshape(MG, P, DC * 4 * P))
        w2t = np.ascontiguousarray(
            w2[e].reshape(MG, 4, P, D).transpose(0, 2, 1, 3)
            .reshape(MG, P, 4 * D))
        de = np.full((C,), TOK, np.int32)  # pad -> trash row (chunk0, row TOK)
        de[:len(ids)] = (ids // TOK) * CHUNK + (ids % TOK)
        perm = np.roll(np.arange(E), -e)   # col 0 = own expert
        in_maps.append({
            "xT": xT_t,
            "xg_t": xg_t,
            "gw": np.ascontiguousarray(gate_w[:, perm]),
            "gb": np.ascontiguousarray(
                np.broadcast_to(gate_b[perm], (P, E))),
            "w1": w1t,
            "b1": np.ascontiguousarray(b1[e]),
            "w2": w2t,
            "b2": np.ascontiguousarray(np.broadcast_to(b2[e], (P, D))),
            "dest": de,
            "ln_g": np.ascontiguousarray(np.broadcast_to(ln_g, (P, D))),
            "ln_b": np.ascontiguousarray(np.broadcast_to(ln_b, (P, D))),
        })

    return C, (Bb, S, D, F, E, T), in_maps


def kernel(x, gate_w, gate_b, w1, b1, w2, b2, ln_g, ln_b):
    C, (Bb, S, D, F, E, T), in_maps = prepare(
        x, gate_w, gate_b, w1, b1, w2, b2, ln_g, ln_b)
    nc = _get_program(C, D, F, E, T)
    res = bass_utils.run_bass_kernel_spmd(
        nc, in_maps, core_ids=list(range(N_CORES)))
    shards = [res.results[c]["out"] for c in range(N_CORES)]
    return np.concatenate(shards, axis=0).reshape(Bb, S, D)


# revision 25
# speedup vs baseline: 1.1175x; 1.0881x over previous
"""MoE forward (top-2 routed, 8 experts) on 8 Trainium2 NeuronCores.

Strategy: expert-parallel. Host computes the (cheap) routing decisions and
gathers each expert's assigned tokens; core e runs its expert's FFN over its
C gathered tokens (fp32r matmuls, fp32 accumulate), computes the gating
softmax on device (fp32) for the gate coefficients, scatters p*(ffn(x)+b2)
rows into a dense [T, D] buffer, then a cross-core ReduceScatter sums the
two expert contributions per token; each core LayerNorms its 512-token shard.
Host concatenates the 8 shards.

Self-contained: hardcodes the problem shapes; compiles per capacity C.
"""
import numpy as np

import concourse.bass as bass
import concourse.bacc as bacc
import concourse.tile as tile
import concourse.mybir as mybir
import concourse.bass_utils as bass_utils
from concourse.bass import IndirectOffsetOnAxis

P = 128
N_CORES = 8
TEMP = 0.9
EPS = 1e-5

f32 = mybir.dt.float32
f32r = mybir.dt.float32r
i32 = mybir.dt.int32
AF = mybir.ActivationFunctionType
OP = mybir.AluOpType


def _sub_blocks(n, pref=512):
    """Split n (multiple of 128) into matmul free-dim blocks <= 512,
    preferring >=256 (full-rate fp32r)."""
    out = []
    rem = n
    while rem > 0:
        if rem == 128 + pref:
            out += [384, 256]
            rem = 0
        elif rem >= pref:
            out.append(pref)
            rem -= pref
        else:
            out.append(rem)
            rem = 0
    return out


def build(C, D=1024, F=4096, E=8, T=4096):
    """Build the SPMD Bass program for capacity C (multiple of 128)."""
    DC, FC = D // P, F // P
    MG = FC // 4                  # m-groups of 4 F-chunks (w1/w2 panel unit)
    NCk = C // P                  # token chunks per core
    TOK = T // N_CORES            # tokens per output shard
    CHUNK = TOK + 1               # +1 trash row per shard chunk
    ZROWS = N_CORES * CHUNK
    LNC = TOK // P                # LayerNorm tiles per shard
    assert D % P == 0 and F % (4 * P) == 0 and TOK % P == 0 and C % P == 0

    nc = bacc.Bacc("TRN2", target_bir_lowering=False, debug=False,
                   enable_asserts=True, num_devices=N_CORES)

    # All big inputs are host pre-tiled to [.., P, free] so each DMA is 128
    # contiguous per-partition descriptors (DMA queues are descriptor-bound
    # otherwise).
    xT = nc.dram_tensor("xT", [P, DC * C], f32, kind="ExternalInput").ap()
    xg_t = nc.dram_tensor("xg_t", [NCk, P, DC * P], f32,
                          kind="ExternalInput").ap()
    gw = nc.dram_tensor("gw", [D, E], f32, kind="ExternalInput").ap()
    gb = nc.dram_tensor("gb", [P, E], f32, kind="ExternalInput").ap()
    w1 = nc.dram_tensor("w1", [MG, P, DC * 4 * P], f32r,
                        kind="ExternalInput").ap()
    b1 = nc.dram_tensor("b1", [F], f32, kind="ExternalInput").ap()
    w2 = nc.dram_tensor("w2", [MG, P, 4 * D], f32r,
                        kind="ExternalInput").ap()
    b2 = nc.dram_tensor("b2", [P, D], f32, kind="ExternalInput").ap()
    # [q, ch, 2]: for shard token ch*P+q, the two source rows (e*C + slot)
    # in the all-gathered contribution tensor.
    gidx = nc.dram_tensor("gidx", [P, LNC * 2], i32, kind="ExternalInput").ap()
    lng = nc.dram_tensor("ln_g", [P, D], f32, kind="ExternalInput").ap()
    lnb = nc.dram_tensor("ln_b", [P, D], f32, kind="ExternalInput").ap()
    out = nc.dram_tensor("out", [TOK, D], f32, kind="ExternalOutput").ap()

    with tile.TileContext(nc) as tc:
        with (
            tc.tile_pool(name="res", bufs=1) as res,          # resident tiles
            tc.tile_pool(name="wpan", bufs=3) as wpan,        # weight panels
            tc.tile_pool(name="hbuf", bufs=1) as hbuf,        # h (gelu out)
            tc.tile_pool(name="work", bufs=2) as work,        # LN / misc
            tc.tile_pool(name="gat", bufs=1) as gat,          # gating temps
            tc.tile_pool(name="ps1", bufs=3, space="PSUM") as ps1p,
            tc.tile_pool(name="ps2", bufs=4, space="PSUM") as ps2p,
            tc.tile_pool(name="dram", bufs=1, space="DRAM") as dram,
        ):
            # ---------------- resident loads ----------------
            x_sb = res.tile([P, DC, C], f32r)
            # gpsimd cast DMAs: round fp32 -> fp32r once here (PE input prep);
            # split per D-chunk so the loads spread across DMA queues.
            for dc in range(DC):
                nc.gpsimd.dma_start(x_sb[:, dc, :], xT[:, dc * C:(dc + 1) * C])
            gw_sb = res.tile([P, DC, E], f32)
            nc.sync.dma_start(gw_sb[:], gw.rearrange("(dc p) e -> p dc e", p=P))
            gb_sb = res.tile([P, E], f32)
            nc.sync.dma_start(gb_sb[:], gb[:, :])
            b1_sb = res.tile([P, FC], f32)
            nc.sync.dma_start(b1_sb[:], b1.rearrange("(fc p) -> p fc", p=P))
            b2_sb = res.tile([P, D], f32)
            nc.sync.dma_start(b2_sb[:], b2[:, :])
            lng_sb = res.tile([P, D], f32)
            nc.sync.dma_start(lng_sb[:], lng[:, :])
            lnb_sb = res.tile([P, D], f32)
            nc.sync.dma_start(lnb_sb[:], lnb[:, :])
            gidx_sb = res.tile([P, LNC, 2], i32)
            nc.sync.dma_start(gidx_sb[:],
                              gidx.rearrange("p (c k) -> p c k", k=2))
            eps_sb = res.tile([P, 1], f32)
            nc.vector.memset(eps_sb[:], EPS)

            # ---------------- gating (fp32) ----------------
            logit = gat.tile([P, NCk, E], f32)
            for t in range(NCk):
                # True-fp32 load of this token chunk straight from DRAM: the
                # fp32r copy x_sb is rounded, and f32r rounding noise in the
                # gating logits can flip top-2 decisions vs the host routing.
                xg = work.tile([P, DC, P], f32, tag="xg")
                nc.sync.dma_start(
                    xg[:], xg_t[t].rearrange("p (dc q) -> p dc q", dc=DC))
                pg = ps1p.tile([P, E], f32, tag="ps1")
                for dc in range(DC):
                    nc.tensor.matmul(
                        pg[:],
                        xg[:, dc, :],
                        gw_sb[:, dc, :],
                        start=(dc == 0), stop=(dc == DC - 1),
                    )
                nc.vector.tensor_add(logit[:, t, :], pg[:], gb_sb[:])

            m1 = gat.tile([P, NCk, 1], f32)
            nc.vector.tensor_reduce(m1[:], logit[:], axis=mybir.AxisListType.X,
                                    op=OP.max)
            tt = gat.tile([P, NCk, E], f32)
            nc.vector.tensor_tensor(tt[:], logit[:],
                                    m1[:].to_broadcast([P, NCk, E]), OP.subtract)
            eq = gat.tile([P, NCk, E], f32)
            nc.vector.tensor_scalar(eq[:], tt[:], 0.0, None, OP.is_equal)
            msk = gat.tile([P, NCk, E], f32)
            nc.vector.scalar_tensor_tensor(msk[:], eq[:], -1e30, tt[:],
                                           OP.mult, OP.add)
            m2 = gat.tile([P, NCk, 1], f32)
            nc.vector.tensor_reduce(m2[:], msk[:], axis=mybir.AxisListType.X,
                                    op=OP.max)
            keep = gat.tile([P, NCk, E], f32)
            nc.vector.tensor_tensor(keep[:], tt[:],
                                    m2[:].to_broadcast([P, NCk, E]), OP.is_ge)
            ex = gat.tile([P, NCk, E], f32)
            nc.scalar.activation(ex[:], tt[:], AF.Exp, scale=float(1.0 / TEMP))
            exk = gat.tile([P, NCk, E], f32)
            nc.vector.tensor_mul(exk[:], ex[:], keep[:])
            ssum = gat.tile([P, NCk, 1], f32)
            nc.vector.tensor_reduce(ssum[:], exk[:], axis=mybir.AxisListType.X,
                                    op=OP.add)
            rcp = gat.tile([P, NCk, 1], f32)
            nc.vector.reciprocal(rcp[:], ssum[:])
            pcols = gat.tile([P, NCk], f32)
            nc.vector.tensor_tensor(pcols[:], exk[:, :, 0],
                                    rcp[:, :, 0], OP.mult)

            # ---------------- y accumulator init: y = p * b2 --------------
            y_acc = res.tile([P, NCk, D], f32)
            for mt in range(NCk):
                nc.vector.tensor_scalar(y_acc[:, mt, :], b2_sb[:],
                                        pcols[:, mt:mt + 1], None, OP.mult)

            # ---------------- main FFN loop ----------------
            nsubs = _sub_blocks(C)
            ndsubs = _sub_blocks(D)
            for mg in range(MG):
                w1p = wpan.tile([P, DC, 4 * P], f32r, tag="wp")
                nc.sync.dma_start(
                    w1p[:], w1[mg].rearrange("p (dc m) -> p dc m", dc=DC))
                w2p = wpan.tile([P, 4, D], f32r, tag="wp")
                nc.sync.dma_start(
                    w2p[:], w2[mg].rearrange("p (mi d) -> p mi d", mi=4))
                h_mg = hbuf.tile([P, 4, C], f32r, tag="h")
                # mm1: h = gelu(x @ w1 + b1), transposed [F-part, tokens]
                for mi in range(4):
                    c0 = 0
                    for ns in nsubs:
                        ps = ps1p.tile([P, ns], f32, tag="ps1")
                        for dc in range(DC):
                            nc.tensor.matmul(
                                ps[:],
                                w1p[:, dc, mi * P:(mi + 1) * P],
                                x_sb[:, dc, c0:c0 + ns],
                                start=(dc == 0), stop=(dc == DC - 1),
                            )
                        nc.scalar.activation(
                            h_mg[:, mi, c0:c0 + ns], ps[:], AF.Gelu,
                            bias=b1_sb[:, mg * 4 + mi:mg * 4 + mi + 1])
                        c0 += ns
                # mm2: y[tok, d] += p * (h.T @ w2)
                for mt in range(NCk):
                    d0 = 0
                    for nd in ndsubs:
                        ps = ps2p.tile([P, nd], f32, tag="ps2")
                        for mi in range(4):
                            nc.tensor.matmul(
                                ps[:],
                                h_mg[:, mi, mt * P:(mt + 1) * P],
                                w2p[:, mi, d0:d0 + nd],
                                start=(mi == 0), stop=(mi == 3),
                            )
                        nc.vector.scalar_tensor_tensor(
                            y_acc[:, mt, d0:d0 + nd], ps[:],
                            pcols[:, mt:mt + 1], y_acc[:, mt, d0:d0 + nd],
                            OP.mult, OP.add)
                        d0 += nd

            # ---------------- ship compact contributions ----------------
            y_out = dram.tile([C, D], f32)
            for mt in range(NCk):
                nc.sync.dma_start(y_out[mt * P:(mt + 1) * P, :],
                                  y_acc[:, mt, :])
            ag = dram.tile([N_CORES * C, D], f32, addr_space="Shared")
            nc.gpsimd.collective_compute(
                "AllGather",
                OP.bypass,
                replica_groups=[list(range(N_CORES))],
                ins=[y_out.opt()],
                outs=[ag.opt()],
            )

            # -------- per-shard combine (2 contributions) + LayerNorm ------
            for ch in range(LNC):
                ga = work.tile([P, D], f32, tag="ln_ga")
                nc.gpsimd.indirect_dma_start(
                    out=ga[:], out_offset=None,
                    in_=ag[:, :],
                    in_offset=IndirectOffsetOnAxis(
                        ap=gidx_sb[:, ch, 0:1], axis=0),
                )
                gb2 = work.tile([P, D], f32, tag="ln_gb")
                nc.gpsimd.indirect_dma_start(
                    out=gb2[:], out_offset=None,
                    in_=ag[:, :],
                    in_offset=IndirectOffsetOnAxis(
                        ap=gidx_sb[:, ch, 1:2], axis=0),
                )
                xt = work.tile([P, D], f32, tag="ln_x")
                nc.vector.tensor_add(xt[:], ga[:], gb2[:])
                s1 = work.tile([P, 1], f32, tag="ln_s1")
                nc.vector.tensor_reduce(s1[:], xt[:], axis=mybir.AxisListType.X,
                                        op=OP.add)
                scr = work.tile([P, D], f32, tag="ln_scr")
                s2 = work.tile([P, 1], f32, tag="ln_s2")
                nc.scalar.activation(scr[:], xt[:], AF.Square, accum_out=s2[:])
                mean = work.tile([P, 1], f32, tag="ln_mean")
                nc.vector.tensor_scalar_mul(mean[:], s1[:], 1.0 / D)
                ex2 = work.tile([P, 1], f32, tag="ln_ex2")
                nc.vector.tensor_scalar_mul(ex2[:], s2[:], 1.0 / D)
                # var = ex2 - mean^2  (computed as (-mean)*mean + ex2)
                var = work.tile([P, 1], f32, tag="ln_var")
                nc.vector.scalar_tensor_tensor(var[:], mean[:], -1.0, mean[:],
                                               OP.mult, OP.mult)
                nc.vector.tensor_add(var[:], var[:], ex2[:])
                sd = work.tile([P, 1], f32, tag="ln_sd")
                nc.scalar.activation(sd[:], var[:], AF.Sqrt, bias=eps_sb[:])
                rstd = work.tile([P, 1], f32, tag="ln_rstd")
                nc.vector.reciprocal(rstd[:], sd[:])
                nrm = work.tile([P, D], f32, tag="ln_nrm")
                nc.vector.tensor_scalar(nrm[:], xt[:], mean[:], rstd[:],
                                        OP.subtract, OP.mult)
                nc.vector.tensor_mul(nrm[:], nrm[:], lng_sb[:])
                nc.vector.tensor_add(nrm[:], nrm[:], lnb_sb[:])
                nc.sync.dma_start(out[ch * P:(ch + 1) * P, :], nrm[:])

    nc.compile()
    return nc


_CACHE = {}


def _get_program(C, D, F, E, T):
    key = (C, D, F, E, T)
    if key not in _CACHE:
        _CACHE[key] = build(C, D=D, F=F, E=E, T=T)
    return _CACHE[key]


def prepare(x, gate_w, gate_b, w1, b1, w2, b2, ln_g, ln_b):
    """Host-side routing + per-core input construction.

    Returns (C, dims, in_maps)."""
    x = np.asarray(x, np.float32)
    gate_w = np.asarray(gate_w, np.float32)
    gate_b = np.asarray(gate_b, np.float32)
    w1 = np.asarray(w1, np.float32)
    b1 = np.asarray(b1, np.float32)
    w2 = np.asarray(w2, np.float32)
    b2 = np.asarray(b2, np.float32)
    ln_g = np.asarray(ln_g, np.float32)
    ln_b = np.asarray(ln_b, np.float32)

    Bb, S, D = x.shape
    E, _, F = w1.shape
    T = Bb * S
    TOK = T // N_CORES
    CHUNK = TOK + 1

    # ---- host routing (decisions only; probabilities are computed on device)
    x2 = np.ascontiguousarray(x.reshape(T, D))
    logits = (x2 @ gate_w + gate_b) / TEMP
    top2 = np.argsort(-logits, axis=1, kind="stable")[:, :2]
    idx = [np.where((top2 == e).any(axis=1))[0] for e in range(E)]
    maxc = max(len(i) for i in idx)
    C = max(((maxc + P - 1) // P) * P, P)

    DC, NCk, MG = D // P, C // P, F // (4 * P)
    LNC = TOK // P
    # rows[t, k] = top2[t,k]*C + slot of t within that expert's batch
    rows = np.empty((T, 2), np.int32)
    for e in range(E):
        ids = idx[e]
        slots = np.arange(len(ids), dtype=np.int32)
        for k in (0, 1):
            m = top2[ids, k] == e
            rows[ids[m], k] = e * C + slots[m]

    in_maps = []
    for e in range(E):
        ids = idx[e]
        xTe = np.zeros((D, C), np.float32)
        xTe[:, :len(ids)] = x2[ids].T
        xT_t = np.ascontiguousarray(
            xTe.reshape(DC, P, C).transpose(1, 0, 2).reshape(P, DC * C))
        xg_t = np.ascontiguousarray(
            xTe.reshape(DC, P, NCk, P).transpose(2, 1, 0, 3)
            .reshape(NCk, P, DC * P))
        w1t = np.ascontiguousarray(
            w1[e].reshape(DC, P, MG, 4 * P).transpose(2, 1, 0, 3)
            .reshape(MG, P, DC * 4 * P))
        w2t = np.ascontiguousarray(
            w2[e].reshape(MG, 4, P, D).transpose(0, 2, 1, 3)
            .reshape(MG, P, 4 * D))
        gidx_core = np.ascontiguousarray(
            rows[e * TOK:(e + 1) * TOK].reshape(LNC, P, 2)
            .transpose(1, 0, 2).reshape(P, LNC * 2))
        perm = np.roll(np.arange(E), -e)   # col 0 = own expert
        in_maps.append({
            "xT": xT_t,
            "xg_t": xg_t,
            "gw": np.ascontiguousarray(gate_w[:, perm]),
            "gb": np.ascontiguousarray(
                np.broadcast_to(gate_b[perm], (P, E))),
            "w1": w1t,
            "b1": np.ascontiguousarray(b1[e]),
            "w2": w2t,
            "b2": np.ascontiguousarray(np.broadcast_to(b2[e], (P, D))),
            "gidx": gidx_core,
            "ln_g": np.ascontiguousarray(np.broadcast_to(ln_g, (P, D))),
            "ln_b": np.ascontiguousarray(np.broadcast_to(ln_b, (P, D))),
        })

    return C, (Bb, S, D, F, E, T), in_maps


def kernel(x, gate_w, gate_b, w1, b1, w2, b2, ln_g, ln_b):
    C, (Bb, S, D, F, E, T), in_maps = prepare(
        x, gate_w, gate_b, w1, b1, w2, b2, ln_g, ln_b)
    nc = _get_program(C, D, F, E, T)
    res = bass_utils.run_bass_kernel_spmd(
        nc, in_maps, core_ids=list(range(N_CORES)))
    shards = [res.results[c]["out"] for c in range(N_CORES)]
    return np.concatenate(shards, axis=0).reshape(Bb, S, D)


# revision 34
# speedup vs baseline: 1.2552x; 1.1233x over previous
"""MoE forward (top-2 routed, 8 experts) on 8 Trainium2 NeuronCores.

Strategy: expert-parallel. Host computes the (cheap) routing decisions and
gathers each expert's assigned tokens; core e runs its expert's FFN over its
C gathered tokens (fp32r matmuls, fp32 accumulate), computes the gating
softmax on device (fp32) for the gate coefficients, scatters p*(ffn(x)+b2)
rows into a dense [T, D] buffer, then a cross-core ReduceScatter sums the
two expert contributions per token; each core LayerNorms its 512-token shard.
Host concatenates the 8 shards.

Self-contained: hardcodes the problem shapes; compiles per capacity C.
"""
import numpy as np

import concourse.bass as bass
import concourse.bacc as bacc
import concourse.tile as tile
import concourse.mybir as mybir
import concourse.bass_utils as bass_utils
from concourse.bass import IndirectOffsetOnAxis

P = 128
N_CORES = 8
TEMP = 0.9
EPS = 1e-5

f32 = mybir.dt.float32
f32r = mybir.dt.float32r
i32 = mybir.dt.int32
AF = mybir.ActivationFunctionType
OP = mybir.AluOpType


def _sub_blocks(n, pref=512):
    """Split n (multiple of 128) into matmul free-dim blocks <= 512,
    preferring >=256 (full-rate fp32r)."""
    out = []
    rem = n
    while rem > 0:
        if rem == 128 + pref:
            out += [384, 256]
            rem = 0
        elif rem >= pref:
            out.append(pref)
            rem -= pref
        else:
            out.append(rem)
            rem = 0
    return out


def build(C, C2, D=1024, F=4096, E=8, T=4096):
    """Build the SPMD Bass program for capacity C (multiple of 128).

    C2 = fixed per-(expert, owner-core) group capacity for the AllToAll."""
    DC, FC = D // P, F // P
    MG = FC // 4                  # m-groups of 4 F-chunks (w1/w2 panel unit)
    NCk = C // P                  # token chunks per core
    TOK = T // N_CORES            # tokens per output shard
    CHUNK = TOK + 1               # +1 trash row per shard chunk
    ZROWS = N_CORES * CHUNK
    LNC = TOK // P                # LayerNorm tiles per shard
    assert D % P == 0 and F % (4 * P) == 0 and TOK % P == 0 and C % P == 0

    nc = bacc.Bacc("TRN2", target_bir_lowering=False, debug=False,
                   enable_asserts=True, num_devices=N_CORES)

    # All big inputs are host pre-tiled to [.., P, free] so each DMA is 128
    # contiguous per-partition descriptors (DMA queues are descriptor-bound
    # otherwise).
    xT = nc.dram_tensor("xT", [P, DC * C], f32, kind="ExternalInput").ap()
    xg_t = nc.dram_tensor("xg_t", [NCk, P, DC * P], f32,
                          kind="ExternalInput").ap()
    gw = nc.dram_tensor("gw", [D, E], f32, kind="ExternalInput").ap()
    gb = nc.dram_tensor("gb", [P, E], f32, kind="ExternalInput").ap()
    w1 = nc.dram_tensor("w1", [MG, P, DC * 4 * P], f32r,
                        kind="ExternalInput").ap()
    b1 = nc.dram_tensor("b1", [F], f32, kind="ExternalInput").ap()
    w2 = nc.dram_tensor("w2", [MG, P, 4 * D], f32r,
                        kind="ExternalInput").ap()
    b2 = nc.dram_tensor("b2", [P, D], f32, kind="ExternalInput").ap()
    # [q, ch, 2]: for shard token ch*P+q, the two source rows (e*C2 + pos)
    # in the post-AllToAll contribution tensor.
    gidx = nc.dram_tensor("gidx", [P, LNC * 2], i32, kind="ExternalInput").ap()
    # [slot]: destination row (owner*C2 + pos) in the pre-AllToAll tensor.
    dest2 = nc.dram_tensor("dest2", [C], i32, kind="ExternalInput").ap()
    lng = nc.dram_tensor("ln_g", [P, D], f32, kind="ExternalInput").ap()
    lnb = nc.dram_tensor("ln_b", [P, D], f32, kind="ExternalInput").ap()
    out = nc.dram_tensor("out", [TOK, D], f32, kind="ExternalOutput").ap()

    with tile.TileContext(nc) as tc:
        with (
            tc.tile_pool(name="res", bufs=1) as res,          # resident tiles
            tc.tile_pool(name="wpan", bufs=3) as wpan,        # weight panels
            tc.tile_pool(name="hbuf", bufs=1) as hbuf,        # h (gelu out)
            tc.tile_pool(name="work", bufs=2) as work,        # LN / misc
            tc.tile_pool(name="gat", bufs=1) as gat,          # gating temps
            tc.tile_pool(name="ps1", bufs=3, space="PSUM") as ps1p,
            tc.tile_pool(name="ps2", bufs=4, space="PSUM") as ps2p,
            tc.tile_pool(name="dram", bufs=1, space="DRAM") as dram,
        ):
            # ---------------- resident loads ----------------
            x_sb = res.tile([P, DC, C], f32r)
            # gpsimd cast DMAs: round fp32 -> fp32r once here (PE input prep);
            # split per D-chunk so the loads spread across DMA queues.
            for dc in range(DC):
                nc.gpsimd.dma_start(x_sb[:, dc, :], xT[:, dc * C:(dc + 1) * C])
            gw_sb = res.tile([P, DC, E], f32)
            nc.sync.dma_start(gw_sb[:], gw.rearrange("(dc p) e -> p dc e", p=P))
            gb_sb = res.tile([P, E], f32)
            nc.sync.dma_start(gb_sb[:], gb[:, :])
            b1_sb = res.tile([P, FC], f32)
            nc.sync.dma_start(b1_sb[:], b1.rearrange("(fc p) -> p fc", p=P))
            b2_sb = res.tile([P, D], f32)
            nc.sync.dma_start(b2_sb[:], b2[:, :])
            lng_sb = res.tile([P, D], f32)
            nc.sync.dma_start(lng_sb[:], lng[:, :])
            lnb_sb = res.tile([P, D], f32)
            nc.sync.dma_start(lnb_sb[:], lnb[:, :])
            gidx_sb = res.tile([P, LNC, 2], i32)
            nc.sync.dma_start(gidx_sb[:],
                              gidx.rearrange("p (c k) -> p c k", k=2))
            dest2_sb = res.tile([P, NCk], i32)
            nc.sync.dma_start(dest2_sb[:], dest2.rearrange("(c p) -> p c", p=P))
            eps_sb = res.tile([P, 1], f32)
            nc.vector.memset(eps_sb[:], EPS)

            # ---------------- gating (fp32) ----------------
            logit = gat.tile([P, NCk, E], f32)
            for t in range(NCk):
                # True-fp32 load of this token chunk straight from DRAM: the
                # fp32r copy x_sb is rounded, and f32r rounding noise in the
                # gating logits can flip top-2 decisions vs the host routing.
                xg = work.tile([P, DC, P], f32, tag="xg")
                nc.sync.dma_start(
                    xg[:], xg_t[t].rearrange("p (dc q) -> p dc q", dc=DC))
                pg = ps1p.tile([P, E], f32, tag="ps1")
                for dc in range(DC):
                    nc.tensor.matmul(
                        pg[:],
                        xg[:, dc, :],
                        gw_sb[:, dc, :],
                        start=(dc == 0), stop=(dc == DC - 1),
                    )
                nc.vector.tensor_add(logit[:, t, :], pg[:], gb_sb[:])

            m1 = gat.tile([P, NCk, 1], f32)
            nc.vector.tensor_reduce(m1[:], logit[:], axis=mybir.AxisListType.X,
                                    op=OP.max)
            tt = gat.tile([P, NCk, E], f32)
            nc.vector.tensor_tensor(tt[:], logit[:],
                                    m1[:].to_broadcast([P, NCk, E]), OP.subtract)
            eq = gat.tile([P, NCk, E], f32)
            nc.vector.tensor_scalar(eq[:], tt[:], 0.0, None, OP.is_equal)
            msk = gat.tile([P, NCk, E], f32)
            nc.vector.scalar_tensor_tensor(msk[:], eq[:], -1e30, tt[:],
                                           OP.mult, OP.add)
            m2 = gat.tile([P, NCk, 1], f32)
            nc.vector.tensor_reduce(m2[:], msk[:], axis=mybir.AxisListType.X,
                                    op=OP.max)
            keep = gat.tile([P, NCk, E], f32)
            nc.vector.tensor_tensor(keep[:], tt[:],
                                    m2[:].to_broadcast([P, NCk, E]), OP.is_ge)
            ex = gat.tile([P, NCk, E], f32)
            nc.scalar.activation(ex[:], tt[:], AF.Exp, scale=float(1.0 / TEMP))
            exk = gat.tile([P, NCk, E], f32)
            nc.vector.tensor_mul(exk[:], ex[:], keep[:])
            ssum = gat.tile([P, NCk, 1], f32)
            nc.vector.tensor_reduce(ssum[:], exk[:], axis=mybir.AxisListType.X,
                                    op=OP.add)
            rcp = gat.tile([P, NCk, 1], f32)
            nc.vector.reciprocal(rcp[:], ssum[:])
            pcols = gat.tile([P, NCk], f32)
            nc.vector.tensor_tensor(pcols[:], exk[:, :, 0],
                                    rcp[:, :, 0], OP.mult)

            # ---------------- y accumulator init: y = p * b2 --------------
            y_acc = res.tile([P, NCk, D], f32)
            for mt in range(NCk):
                nc.vector.tensor_scalar(y_acc[:, mt, :], b2_sb[:],
                                        pcols[:, mt:mt + 1], None, OP.mult)

            # ---------------- main FFN loop ----------------
            nsubs = _sub_blocks(C)
            ndsubs = _sub_blocks(D)
            for mg in range(MG):
                w1p = wpan.tile([P, DC, 4 * P], f32r, tag="wp")
                nc.sync.dma_start(
                    w1p[:], w1[mg].rearrange("p (dc m) -> p dc m", dc=DC))
                w2p = wpan.tile([P, 4, D], f32r, tag="wp")
                nc.sync.dma_start(
                    w2p[:], w2[mg].rearrange("p (mi d) -> p mi d", mi=4))
                h_mg = hbuf.tile([P, 4, C], f32r, tag="h")
                # mm1: h = gelu(x @ w1 + b1), transposed [F-part, tokens]
                for mi in range(4):
                    c0 = 0
                    for ns in nsubs:
                        ps = ps1p.tile([P, ns], f32, tag="ps1")
                        for dc in range(DC):
                            nc.tensor.matmul(
                                ps[:],
                                w1p[:, dc, mi * P:(mi + 1) * P],
                                x_sb[:, dc, c0:c0 + ns],
                                start=(dc == 0), stop=(dc == DC - 1),
                            )
                        nc.scalar.activation(
                            h_mg[:, mi, c0:c0 + ns], ps[:], AF.Gelu,
                            bias=b1_sb[:, mg * 4 + mi:mg * 4 + mi + 1])
                        c0 += ns
                # mm2: y[tok, d] += p * (h.T @ w2)
                for mt in range(NCk):
                    d0 = 0
                    for nd in ndsubs:
                        ps = ps2p.tile([P, nd], f32, tag="ps2")
                        for mi in range(4):
                            nc.tensor.matmul(
                                ps[:],
                                h_mg[:, mi, mt * P:(mt + 1) * P],
                                w2p[:, mi, d0:d0 + nd],
                                start=(mi == 0), stop=(mi == 3),
                            )
                        nc.vector.scalar_tensor_tensor(
                            y_acc[:, mt, d0:d0 + nd], ps[:],
                            pcols[:, mt:mt + 1], y_acc[:, mt, d0:d0 + nd],
                            OP.mult, OP.add)
                        d0 += nd

            # ------- ship contributions grouped by owner core (AllToAll) ----
            y_a2a = dram.tile([N_CORES * C2, D], f32)
            for mt in range(NCk):
                nc.gpsimd.indirect_dma_start(
                    out=y_a2a[:, :],
                    out_offset=IndirectOffsetOnAxis(
                        ap=dest2_sb[:, mt:mt + 1], axis=0),
                    in_=y_acc[:, mt, :],
                    in_offset=None,
                )
            ag = dram.tile([N_CORES * C2, D], f32)
            nc.gpsimd.collective_compute(
                "AllToAll",
                OP.bypass,
                replica_groups=[list(range(N_CORES))],
                ins=[y_a2a.opt()],
                outs=[ag.opt()],
            )

            # -------- per-shard combine (2 contributions) + LayerNorm ------
            for ch in range(LNC):
                ga = work.tile([P, D], f32, tag="ln_ga")
                nc.gpsimd.indirect_dma_start(
                    out=ga[:], out_offset=None,
                    in_=ag[:, :],
                    in_offset=IndirectOffsetOnAxis(
                        ap=gidx_sb[:, ch, 0:1], axis=0),
                )
                gb2 = work.tile([P, D], f32, tag="ln_gb")
                nc.gpsimd.indirect_dma_start(
                    out=gb2[:], out_offset=None,
                    in_=ag[:, :],
                    in_offset=IndirectOffsetOnAxis(
                        ap=gidx_sb[:, ch, 1:2], axis=0),
                )
                xt = work.tile([P, D], f32, tag="ln_x")
                nc.vector.tensor_add(xt[:], ga[:], gb2[:])
                s1 = work.tile([P, 1], f32, tag="ln_s1")
                nc.vector.tensor_reduce(s1[:], xt[:], axis=mybir.AxisListType.X,
                                        op=OP.add)
                scr = work.tile([P, D], f32, tag="ln_scr")
                s2 = work.tile([P, 1], f32, tag="ln_s2")
                nc.scalar.activation(scr[:], xt[:], AF.Square, accum_out=s2[:])
                mean = work.tile([P, 1], f32, tag="ln_mean")
                nc.vector.tensor_scalar_mul(mean[:], s1[:], 1.0 / D)
                ex2 = work.tile([P, 1], f32, tag="ln_ex2")
                nc.vector.tensor_scalar_mul(ex2[:], s2[:], 1.0 / D)
                # var = ex2 - mean^2  (computed as (-mean)*mean + ex2)
                var = work.tile([P, 1], f32, tag="ln_var")
                nc.vector.scalar_tensor_tensor(var[:], mean[:], -1.0, mean[:],
                                               OP.mult, OP.mult)
                nc.vector.tensor_add(var[:], var[:], ex2[:])
                sd = work.tile([P, 1], f32, tag="ln_sd")
                nc.scalar.activation(sd[:], var[:], AF.Sqrt, bias=eps_sb[:])
                rstd = work.tile([P, 1], f32, tag="ln_rstd")
                nc.vector.reciprocal(rstd[:], sd[:])
                nrm = work.tile([P, D], f32, tag="ln_nrm")
                nc.vector.tensor_scalar(nrm[:], xt[:], mean[:], rstd[:],
                                        OP.subtract, OP.mult)
                nc.vector.tensor_mul(nrm[:], nrm[:], lng_sb[:])
                nc.vector.tensor_add(nrm[:], nrm[:], lnb_sb[:])
                nc.sync.dma_start(out[ch * P:(ch + 1) * P, :], nrm[:])

    nc.compile()
    return nc


_CACHE = {}


def _get_program(C, C2, D, F, E, T):
    key = (C, C2, D, F, E, T)
    if key not in _CACHE:
        _CACHE[key] = build(C, C2, D=D, F=F, E=E, T=T)
    return _CACHE[key]


def prepare(x, gate_w, gate_b, w1, b1, w2, b2, ln_g, ln_b):
    """Host-side routing + per-core input construction.

    Returns (C, dims, in_maps)."""
    x = np.asarray(x, np.float32)
    gate_w = np.asarray(gate_w, np.float32)
    gate_b = np.asarray(gate_b, np.float32)
    w1 = np.asarray(w1, np.float32)
    b1 = np.asarray(b1, np.float32)
    w2 = np.asarray(w2, np.float32)
    b2 = np.asarray(b2, np.float32)
    ln_g = np.asarray(ln_g, np.float32)
    ln_b = np.asarray(ln_b, np.float32)

    Bb, S, D = x.shape
    E, _, F = w1.shape
    T = Bb * S
    TOK = T // N_CORES
    CHUNK = TOK + 1

    # ---- host routing (decisions only; probabilities are computed on device)
    x2 = np.ascontiguousarray(x.reshape(T, D))
    logits = (x2 @ gate_w + gate_b) / TEMP
    top2 = np.argsort(-logits, axis=1, kind="stable")[:, :2]
    idx = [np.where((top2 == e).any(axis=1))[0] for e in range(E)]
    maxc = max(len(i) for i in idx)
    C = max(((maxc + P - 1) // P) * P, P)

    DC, NCk, MG = D // P, C // P, F // (4 * P)
    LNC = TOK // P

    # Owner-core grouping for the AllToAll: within each expert batch (ids
    # ascending), tokens are contiguous runs per owner core j = t // TOK.
    cnts = np.stack([np.bincount(idx[e] // TOK, minlength=N_CORES)
                     for e in range(E)])               # [E, cores]
    C2 = int(((cnts.max() + 15) // 16) * 16)
    if (cnts == C2).any():
        C2 += 16  # guarantee a free trash row in some chunk on every core

    # rows[t, k] = top2[t,k]*C2 + position of t within that (expert, owner)
    # group — the source row on the owner core after the AllToAll.
    rows = np.empty((T, 2), np.int32)
    dest2s = []
    for e in range(E):
        ids = idx[e]
        owner = ids // TOK
        gstart = np.concatenate([[0], np.cumsum(cnts[e])[:-1]])
        pos = np.arange(len(ids), dtype=np.int32) - gstart[owner].astype(np.int32)
        d2 = np.full((C,), 0, np.int32)
        d2[:len(ids)] = owner.astype(np.int32) * C2 + pos
        # pads -> a guaranteed-unused row (some chunk with count < C2)
        jfree = int(np.argmin(cnts[e]))
        d2[len(ids):] = jfree * C2 + C2 - 1
        dest2s.append(d2)
        for k in (0, 1):
            m = top2[ids, k] == e
            rows[ids[m], k] = e * C2 + pos[m]

    in_maps = []
    for e in range(E):
        ids = idx[e]
        xTe = np.zeros((D, C), np.float32)
        xTe[:, :len(ids)] = x2[ids].T
        xT_t = np.ascontiguousarray(
            xTe.reshape(DC, P, C).transpose(1, 0, 2).reshape(P, DC * C))
        xg_t = np.ascontiguousarray(
            xTe.reshape(DC, P, NCk, P).transpose(2, 1, 0, 3)
            .reshape(NCk, P, DC * P))
        w1t = np.ascontiguousarray(
            w1[e].reshape(DC, P, MG, 4 * P).transpose(2, 1, 0, 3)
            .reshape(MG, P, DC * 4 * P))
        w2t = np.ascontiguousarray(
            w2[e].reshape(MG, 4, P, D).transpose(0, 2, 1, 3)
            .reshape(MG, P, 4 * D))
        gidx_core = np.ascontiguousarray(
            rows[e * TOK:(e + 1) * TOK].reshape(LNC, P, 2)
            .transpose(1, 0, 2).reshape(P, LNC * 2))
        perm = np.roll(np.arange(E), -e)   # col 0 = own expert
        in_maps.append({
            "xT": xT_t,
            "xg_t": xg_t,
            "gw": np.ascontiguousarray(gate_w[:, perm]),
            "gb": np.ascontiguousarray(
                np.broadcast_to(gate_b[perm], (P, E))),
            "w1": w1t,
            "b1": np.ascontiguousarray(b1[e]),
            "w2": w2t,
            "b2": np.ascontiguousarray(np.broadcast_to(b2[e], (P, D))),
            "gidx": gidx_core,
            "dest2": dest2s[e],
            "ln_g": np.ascontiguousarray(np.broadcast_to(ln_g, (P, D))),
            "ln_b": np.ascontiguousarray(np.broadcast_to(ln_b, (P, D))),
        })

    return C, C2, (Bb, S, D, F, E, T), in_maps


def kernel(x, gate_w, gate_b, w1, b1, w2, b2, ln_g, ln_b):
    C, C2, (Bb, S, D, F, E, T), in_maps = prepare(
        x, gate_w, gate_b, w1, b1, w2, b2, ln_g, ln_b)
    nc = _get_program(C, C2, D, F, E, T)
    res = bass_utils.run_bass_kernel_spmd(
        nc, in_maps, core_ids=list(range(N_CORES)))
    shards = [res.results[c]["out"] for c in range(N_CORES)]
    return np.concatenate(shards, axis=0).reshape(Bb, S, D)
